# revision 9
# baseline (speedup 1.0000x reference)
"""nn_AttSeqM_67748814127286 — data-parallel Bass kernel across 8 NeuronCores.

Host side: shards batch (2048 -> 8 x 256), builds x = concat(qcv, posembed[posid])
in bf16 row-major, plus a small feature-major q-input slice. Device side (per
core): xbar-transposed load of x -> xT [128, rows]; gated projections via bf16
PE matmuls (k feature-major, v row-major); LayerNorm via per-row accumulated
stats; attention with per-b score/ctx matmuls using unnormalized exp weights.
Softmax denominators are returned separately and divided out on the host.

Falls back to a numpy forward if inputs deviate from the expected structure
(non-zero biases / non-trivial mask / LN affine), so correctness never regresses.
"""
import sys
import numpy as np

if "/opt/trn_rl_repo" not in sys.path:
    sys.path.insert(0, "/opt/trn_rl_repo")

B, S, INQ = 2048, 200, 120
POS_E = 8
H, QLEN, VLEN = 8, 16, 64
HID = H * VLEN          # 512
IN_F = INQ + POS_E      # 128
LN_EPS = 1e-5
N_CORES = 8
NB = B // N_CORES       # 256 batch rows per core
R = NB * S              # 51200 x-rows per core
CHUNK_B = 16            # batch rows processed per chunk
NCH = NB // CHUNK_B     # 16 chunks per core

_CACHE = {}


# ---------------------------------------------------------------- host helpers

def _to_bf16(a):
    """fp32 ndarray -> ml_dtypes.bfloat16 ndarray (round to nearest even)."""
    import ml_dtypes
    a = np.ascontiguousarray(a, dtype=np.float32)
    u = a.view(np.uint32)
    r = ((u + 0x7FFF + ((u >> 16) & 1)) >> 16).astype(np.uint16)
    return r.view(ml_dtypes.bfloat16).reshape(a.shape)


def _forward_np(posid, qcv, mask, posembed, Wq, bq, Wqc, bqc, Wk, bk, Wkc, bkc,
                Wv, bv, Wvc, bvc, v_ln_g, v_ln_b):
    def sigmoid(z):
        return 1.0 / (1.0 + np.exp(-z))

    def css(x, W, b, Wc, bc):
        return (x @ W + b) * sigmoid(x @ Wc + bc)

    def layernorm(x, g, b):
        mu = x.mean(-1, keepdims=True)
        var = x.var(-1, keepdims=True)
        return (x - mu) / np.sqrt(var + LN_EPS) * g + b

    Bq = posid.shape[0]
    pe = posembed[posid]
    x = np.concatenate([qcv, pe], axis=-1).astype(np.float32)

    q = css(x[:, 0:1], Wq, bq, Wqc, bqc)
    k = css(x, Wk, bk, Wkc, bkc)
    v = layernorm(css(x, Wv, bv, Wvc, bvc), v_ln_g, v_ln_b)

    q = q.reshape(Bq, 1, H, QLEN).transpose(0, 2, 1, 3)
    k = k.reshape(Bq, S, H, QLEN).transpose(0, 2, 1, 3)
    v = v.reshape(Bq, S, H, VLEN).transpose(0, 2, 1, 3)

    mask_add = (1.0 - mask) * -10000.0
    scores = np.einsum('bhqd,bhkd->bhqk', q, k)
    scores = (scores + mask_add[None, None, None, :]) / np.float32(np.sqrt(QLEN))
    scores = scores - scores.max(-1, keepdims=True)
    e = np.exp(scores)
    probs = e / e.sum(-1, keepdims=True)
    ctx = np.einsum('bhqk,bhkd->bhqd', probs, v)
    return ctx.transpose(0, 2, 1, 3).reshape(Bq, 1, HID).astype(np.float32)


def _is_lean(inputs):
    """True when biases are zero, mask is all-ones and LN affine is trivial."""
    z = lambda a: not np.any(np.asarray(a))
    return (z(inputs["bq"]) and z(inputs["bqc"]) and z(inputs["bk"])
            and z(inputs["bkc"]) and z(inputs["bv"]) and z(inputs["bvc"])
            and z(inputs["v_ln_b"])
            and np.all(np.asarray(inputs["mask"]) == 1.0)
            and np.all(np.asarray(inputs["v_ln_g"]) == 1.0))


# ---------------------------------------------------------------- bass builder

def _build_nc(nb, chunk_b):
    import concourse.bass as bass
    import concourse.tile as tile
    from concourse import mybir

    bf16 = mybir.dt.bfloat16
    f32 = mybir.dt.float32
    AF = mybir.ActivationFunctionType
    OP = mybir.AluOpType

    nch = nb // chunk_b
    crows = chunk_b * S
    nsub = crows // 400          # k-projection N=400 sub-chunks

    nc = bass.Bass("TRN2", target_bir_lowering=False, debug=False)

    x_d = nc.dram_tensor("x", [nb * S, IN_F], bf16, kind="ExternalInput").ap()
    xq_d = nc.dram_tensor("xq", [IN_F, nb], bf16, kind="ExternalInput").ap()
    wq_d = nc.dram_tensor("wq", [IN_F, H * QLEN], bf16, kind="ExternalInput").ap()
    wqc_d = nc.dram_tensor("wqc", [IN_F, H * QLEN], bf16, kind="ExternalInput").ap()
    wk_d = nc.dram_tensor("wk", [IN_F, H * QLEN], bf16, kind="ExternalInput").ap()
    wkc_d = nc.dram_tensor("wkc", [IN_F, H * QLEN], bf16, kind="ExternalInput").ap()
    wv_d = nc.dram_tensor("wv", [IN_F, HID], bf16, kind="ExternalInput").ap()
    wvc_d = nc.dram_tensor("wvc", [IN_F, HID], bf16, kind="ExternalInput").ap()
    ctxo_d = nc.dram_tensor("ctxo", [nb, H, HID], bf16, kind="ExternalOutput").ap()
    dout_d = nc.dram_tensor("dout", [nch, H * chunk_b], f32,
                            kind="ExternalOutput").ap()

    with tile.TileContext(nc) as tc:
        from contextlib import ExitStack
        with ExitStack() as ctx:
            consts = ctx.enter_context(tc.tile_pool(name="consts", bufs=1))
            xpool = ctx.enter_context(tc.tile_pool(name="xT", bufs=2))
            kpool = ctx.enter_context(tc.tile_pool(name="kT", bufs=2))
            vgpool = ctx.enter_context(tc.tile_pool(name="vg", bufs=2))
            epool = ctx.enter_context(tc.tile_pool(name="e", bufs=2))
            scr = ctx.enter_context(tc.tile_pool(name="scr", bufs=3))
            stats = ctx.enter_context(tc.tile_pool(name="stats", bufs=2))
            ctxp = ctx.enter_context(tc.tile_pool(name="ctxsb", bufs=2))
            qb = ctx.enter_context(tc.tile_pool(name="qblk", bufs=1))
            # PSUM: proj 4 banks + sc 2 + d 1 + ctx 1 = 8
            psproj = ctx.enter_context(tc.tile_pool(name="psproj", bufs=4, space="PSUM"))
            pssc = ctx.enter_context(tc.tile_pool(name="pssc", bufs=2, space="PSUM"))
            psd = ctx.enter_context(tc.tile_pool(name="psd", bufs=1, space="PSUM"))
            psctx = ctx.enter_context(tc.tile_pool(name="psctx", bufs=1, space="PSUM"))

            # ---- constants
            wk = consts.tile([IN_F, 128], bf16, tag="wk")
            wkc = consts.tile([IN_F, 128], bf16, tag="wkc")
            wv = consts.tile([IN_F, HID], bf16, tag="wv")
            wvc = consts.tile([IN_F, HID], bf16, tag="wvc")
            wq = consts.tile([IN_F, 128], bf16, tag="wq")
            wqc = consts.tile([IN_F, 128], bf16, tag="wqc")
            xq = consts.tile([IN_F, nb], bf16, tag="xq")
            nc.sync.dma_start(out=wk, in_=wk_d)
            nc.sync.dma_start(out=wkc, in_=wkc_d)
            nc.sync.dma_start(out=wv, in_=wv_d)
            nc.sync.dma_start(out=wvc, in_=wvc_d)
            nc.sync.dma_start(out=wq, in_=wq_d)
            nc.sync.dma_start(out=wqc, in_=wqc_d)
            nc.sync.dma_start(out=xq, in_=xq_d)

            ones_col = consts.tile([128, 1], bf16, tag="ones")
            nc.vector.memset(ones_col, 1.0)
            eps_col = consts.tile([128, 1], f32, tag="eps")
            nc.vector.memset(eps_col, LN_EPS)

            blkmask = consts.tile([128, H], bf16, tag="blkmask")
            nc.gpsimd.memset(blkmask, 1.0)
            # keep 1 where 0 <= p - 16*j <= 15 else 0
            nc.gpsimd.affine_select(
                out=blkmask, in_=blkmask, compare_op=OP.is_ge, fill=0.0,
                base=0, pattern=[[-QLEN, H]], channel_multiplier=1)
            nc.gpsimd.affine_select(
                out=blkmask, in_=blkmask, compare_op=OP.is_ge, fill=0.0,
                base=QLEN - 1, pattern=[[QLEN, H]], channel_multiplier=-1)

            # ---- q projection (feature-major), fold 1/sqrt(QLEN)=0.25 in
            qps = psproj.tile([128, nb], f32, tag="proj")
            qcps = psproj.tile([128, nb], f32, tag="proj")
            nc.tensor.matmul(qps, lhsT=wq, rhs=xq, start=True, stop=True)
            nc.tensor.matmul(qcps, lhsT=wqc, rhs=xq, start=True, stop=True)
            qsig = scr.tile([128, nb], bf16, tag="qsig")
            nc.scalar.activation(qsig, qcps, AF.Sigmoid)
            qgT = consts.tile([128, nb], f32, tag="qgT")
            nc.vector.scalar_tensor_tensor(
                out=qgT, in0=qps, scalar=0.25, in1=qsig,
                op0=OP.mult, op1=OP.mult)

            # block-diagonal q for the score matmuls
            qblk = qb.tile([128, nb, H], bf16, tag="qblk")
            for b in range(nb):
                nc.vector.tensor_scalar_mul(
                    out=qblk[:, b, :], in0=blkmask, scalar1=qgT[:, b:b + 1])

            # ---- main loop over chunks
            for c in range(nch):
                xT = xpool.tile([IN_F, crows], bf16, tag="xT")
                nc.sync.dma_start_transpose(
                    out=xT, in_=x_d[c * crows:(c + 1) * crows, :])

                # k projection, feature-major [128 f, crows]
                kT = kpool.tile([128, crows], bf16, tag="kT")
                for sub in range(nsub):
                    sl = slice(sub * 400, (sub + 1) * 400)
                    kps = psproj.tile([128, 400], f32, tag="proj")
                    kcps = psproj.tile([128, 400], f32, tag="proj")
                    nc.tensor.matmul(kps, lhsT=wk, rhs=xT[:, sl], start=True, stop=True)
                    nc.tensor.matmul(kcps, lhsT=wkc, rhs=xT[:, sl], start=True, stop=True)
                    ksig = scr.tile([128, 400], bf16, tag="ksig")
                    nc.scalar.activation(ksig, kcps, AF.Sigmoid)
                    nc.vector.tensor_mul(out=kT[:, sl], in0=kps, in1=ksig)

                # v projection row-major per (b, piece), gate + stats
                vg1 = vgpool.tile([128, chunk_b, HID], bf16, tag="vg1")
                vg2 = vgpool.tile([128, chunk_b, HID], bf16, tag="vg2")
                sums = stats.tile([128, 2 * chunk_b], f32, tag="sums")
                ssq = stats.tile([128, 2 * chunk_b], f32, tag="ssq")
                nc.vector.memset(sums, 0.0)
                nc.vector.memset(ssq, 0.0)
                for b in range(chunk_b):
                    for pi, (po, L) in enumerate(((0, 128), (128, 72))):
                        col = pi * chunk_b + b
                        xsl = xT[:, b * S + po: b * S + po + L]
                        vps = psproj.tile([128, HID], f32, tag="proj")
                        vcps = psproj.tile([128, HID], f32, tag="proj")
                        nc.tensor.matmul(vps[0:L, :], lhsT=xsl, rhs=wv,
                                         start=True, stop=True)
                        nc.tensor.matmul(vcps[0:L, :], lhsT=xsl, rhs=wvc,
                                         start=True, stop=True)
                        vsig = scr.tile([128, HID], bf16, tag="vsig")
                        nc.scalar.activation(vsig[0:L, :], vcps[0:L, :], AF.Sigmoid)
                        vg = vg1 if pi == 0 else vg2
                        nc.vector.scalar_tensor_tensor(
                            out=vg[0:L, b, :], in0=vps[0:L, :], scalar=0.0,
                            in1=vsig[0:L, :], op0=OP.add, op1=OP.mult,
                            accum_out=sums[0:L, col:col + 1])
                        sq = scr.tile([128, HID], bf16, tag="sq")
                        nc.scalar.activation(
                            sq[0:L, :], vg[0:L, b, :], AF.Square,
                            accum_out=ssq[0:L, col:col + 1])

                # LayerNorm stats for the whole chunk
                mu = stats.tile([128, 2 * chunk_b], f32, tag="mu")
                mu2 = stats.tile([128, 2 * chunk_b], f32, tag="mu2")
                var = stats.tile([128, 2 * chunk_b], f32, tag="var")
                rstd = stats.tile([128, 2 * chunk_b], f32, tag="rstd")
                nc.vector.tensor_scalar_mul(out=mu, in0=sums, scalar1=1.0 / HID)
                nc.vector.tensor_mul(out=mu2, in0=mu, in1=mu)
                nc.vector.scalar_tensor_tensor(
                    out=var, in0=ssq, scalar=1.0 / HID, in1=mu2,
                    op0=OP.mult, op1=OP.subtract)
                nc.scalar.activation(rstd, var, AF.Sqrt, bias=eps_col)
                nc.vector.reciprocal(out=rstd, in_=rstd)

                # normalize vg in place: (vg - mu) * rstd
                for b in range(chunk_b):
                    for pi, (po, L) in enumerate(((0, 128), (128, 72))):
                        col = pi * chunk_b + b
                        vg = vg1 if pi == 0 else vg2
                        nc.vector.tensor_scalar(
                            out=vg[0:L, b, :], in0=vg[0:L, b, :],
                            scalar1=mu[0:L, col:col + 1],
                            scalar2=rstd[0:L, col:col + 1],
                            op0=OP.subtract, op1=OP.mult)

                # scores (transposed): [s, 8] per b packed into [*, 8*chunk_b]
                sc1 = pssc.tile([128, H * chunk_b], f32, tag="sc")
                sc2 = pssc.tile([128, H * chunk_b], f32, tag="sc")
                for b in range(chunk_b):
                    nc.tensor.matmul(
                        sc1[:, H * b:H * (b + 1)],
                        lhsT=kT[:, b * S:b * S + 128],
                        rhs=qblk[:, c * chunk_b + b, :], start=True, stop=True)
                    nc.tensor.matmul(
                        sc2[0:72, H * b:H * (b + 1)],
                        lhsT=kT[:, b * S + 128:b * S + 200],
                        rhs=qblk[:, c * chunk_b + b, :], start=True, stop=True)
                e1 = epool.tile([128, H * chunk_b], bf16, tag="e1")
                e2 = epool.tile([128, H * chunk_b], bf16, tag="e2")
                nc.scalar.activation(e1, sc1, AF.Exp)
                nc.scalar.activation(e2[0:72, :], sc2[0:72, :], AF.Exp)

                # softmax denominators: D[8b+h] = sum_s e
                m = H * chunk_b
                dps = psd.tile([128, 1], f32, tag="d")
                nc.tensor.matmul(dps[0:m, :], lhsT=e1, rhs=ones_col,
                                 start=True, stop=False)
                nc.tensor.matmul(dps[0:m, :], lhsT=e2[0:72, :],
                                 rhs=ones_col[0:72, :], start=False, stop=True)
                dsb = stats.tile([128, 1], f32, tag="dsb")
                nc.scalar.copy(dsb[0:m, :], dps[0:m, :])
                nc.sync.dma_start(out=dout_d[c, :], in_=dsb[0:m, :])

                # ctx: [8, 512] per b, unnormalized
                ctxsb = ctxp.tile([H, chunk_b, HID], bf16, tag="ctxsb")
                for b in range(chunk_b):
                    cps = psctx.tile([H, HID], f32, tag="ctx")
                    nc.tensor.matmul(cps, lhsT=e1[:, H * b:H * (b + 1)],
                                     rhs=vg1[:, b, :], start=True, stop=False)
                    nc.tensor.matmul(cps, lhsT=e2[0:72, H * b:H * (b + 1)],
                                     rhs=vg2[0:72, b, :], start=False, stop=True)
                    if b % 2 == 0:
                        nc.scalar.copy(ctxsb[:, b, :], cps)
                    else:
                        nc.vector.tensor_copy(out=ctxsb[:, b, :], in_=cps)
                nc.sync.dma_start(
                    out=ctxo_d[c * chunk_b:(c + 1) * chunk_b, :, :].rearrange(
                        "b h f -> h b f"),
                    in_=ctxsb)

    return nc


# ---------------------------------------------------------------- host driver

def _prep_core_inputs(inputs, nb, n_cores):
    """Build per-core in_maps (bf16 x, xq, weights)."""
    import ml_dtypes
    posid = np.asarray(inputs["posid"])
    if posid.dtype != np.int64 and posid.dtype != np.int32:
        posid = posid.astype(np.int32)
    qcv = np.asarray(inputs["qcv"], dtype=np.float32)
    posembed_bf = _to_bf16(np.asarray(inputs["posembed"], dtype=np.float32))

    ntot = posid.shape[0] * posid.shape[1]
    x = np.empty((ntot, IN_F), dtype=ml_dtypes.bfloat16)
    x[:, :INQ] = _to_bf16(qcv.reshape(ntot, INQ))
    x[:, INQ:] = posembed_bf[posid.reshape(ntot)]

    w = {n: np.ascontiguousarray(_to_bf16(np.asarray(inputs[k], np.float32)))
         for n, k in (("wq", "Wq"), ("wqc", "Wqc"), ("wk", "Wk"),
                      ("wkc", "Wkc"), ("wv", "Wv"), ("wvc", "Wvc"))}

    rows = nb * S
    in_maps = []
    for core in range(n_cores):
        xc = x[core * rows:(core + 1) * rows]
        xqc = np.ascontiguousarray(xc[0::S][:nb].T)    # [128, nb]
        m = {"x": xc, "xq": xqc}
        m.update(w)
        in_maps.append(m)
    return in_maps


def _run_device(inputs):
    from concourse.bass_utils import run_bass_kernel_spmd

    key = "nc"
    if key not in _CACHE:
        _CACHE[key] = _build_nc(NB, CHUNK_B)
    nc = _CACHE[key]

    in_maps = _prep_core_inputs(inputs, NB, N_CORES)
    res = run_bass_kernel_spmd(nc, in_maps, core_ids=list(range(N_CORES)))

    outs = []
    for core in range(N_CORES):
        r = res.results[core]
        ctxo = np.asarray(r["ctxo"], dtype=np.float32)       # [nb, 8, 512]
        d = np.asarray(r["dout"], dtype=np.float32)          # [nch, 8*chunk_b]
        d = d.reshape(NCH, CHUNK_B, H).reshape(NB, H)
        hh = np.arange(H)
        diag = ctxo.reshape(NB, H, H, VLEN)[:, hh, hh, :]    # [nb, H, VLEN]
        ctx = diag / d[:, :, None]
        outs.append(ctx.reshape(NB, 1, HID))
    return np.concatenate(outs, axis=0).astype(np.float32)


def kernel(**inputs) -> np.ndarray:
    args = {k: np.asarray(v) for k, v in inputs.items()}
    for k, v in args.items():
        if v.dtype == np.float64:
            args[k] = v.astype(np.float32)
    if not _is_lean(args):
        return _forward_np(**args)
    try:
        return _run_device(args)
    except Exception:
        import traceback
        traceback.print_exc()
        return _forward_np(**args)


# revision 11
# speedup vs baseline: 28.1080x; 28.1080x over previous
"""nn_AttSeqM_67748814127286 — data-parallel Bass kernel across 8 NeuronCores.

Host side: shards batch (2048 -> 8 x 256), builds x = concat(qcv, posembed[posid])
in bf16 row-major, plus a small feature-major q-input slice. Device side (per
core): xbar-transposed load of x -> xT [128, rows]; gated projections via bf16
PE matmuls (k feature-major, v row-major); LayerNorm via per-row accumulated
stats; attention with per-b score/ctx matmuls using unnormalized exp weights.
Softmax denominators are returned separately and divided out on the host.

Falls back to a numpy forward if inputs deviate from the expected structure
(non-zero biases / non-trivial mask / LN affine), so correctness never regresses.
"""
import sys
import numpy as np

if "/opt/trn_rl_repo" not in sys.path:
    sys.path.insert(0, "/opt/trn_rl_repo")

B, S, INQ = 2048, 200, 120
POS_E = 8
H, QLEN, VLEN = 8, 16, 64
HID = H * VLEN          # 512
IN_F = INQ + POS_E      # 128
LN_EPS = 1e-5
N_CORES = 8
NB = B // N_CORES       # 256 batch rows per core
R = NB * S              # 51200 x-rows per core
CHUNK_B = 16            # batch rows processed per chunk
NCH = NB // CHUNK_B     # 16 chunks per core

_CACHE = {}


# ---------------------------------------------------------------- host helpers

def _to_bf16(a):
    """fp32 ndarray -> ml_dtypes.bfloat16 ndarray (round to nearest even)."""
    import ml_dtypes
    a = np.ascontiguousarray(a, dtype=np.float32)
    u = a.view(np.uint32)
    r = ((u + 0x7FFF + ((u >> 16) & 1)) >> 16).astype(np.uint16)
    return r.view(ml_dtypes.bfloat16).reshape(a.shape)


def _forward_np(posid, qcv, mask, posembed, Wq, bq, Wqc, bqc, Wk, bk, Wkc, bkc,
                Wv, bv, Wvc, bvc, v_ln_g, v_ln_b):
    def sigmoid(z):
        return 1.0 / (1.0 + np.exp(-z))

    def css(x, W, b, Wc, bc):
        return (x @ W + b) * sigmoid(x @ Wc + bc)

    def layernorm(x, g, b):
        mu = x.mean(-1, keepdims=True)
        var = x.var(-1, keepdims=True)
        return (x - mu) / np.sqrt(var + LN_EPS) * g + b

    Bq = posid.shape[0]
    pe = posembed[posid]
    x = np.concatenate([qcv, pe], axis=-1).astype(np.float32)

    q = css(x[:, 0:1], Wq, bq, Wqc, bqc)
    k = css(x, Wk, bk, Wkc, bkc)
    v = layernorm(css(x, Wv, bv, Wvc, bvc), v_ln_g, v_ln_b)

    q = q.reshape(Bq, 1, H, QLEN).transpose(0, 2, 1, 3)
    k = k.reshape(Bq, S, H, QLEN).transpose(0, 2, 1, 3)
    v = v.reshape(Bq, S, H, VLEN).transpose(0, 2, 1, 3)

    mask_add = (1.0 - mask) * -10000.0
    scores = np.einsum('bhqd,bhkd->bhqk', q, k)
    scores = (scores + mask_add[None, None, None, :]) / np.float32(np.sqrt(QLEN))
    scores = scores - scores.max(-1, keepdims=True)
    e = np.exp(scores)
    probs = e / e.sum(-1, keepdims=True)
    ctx = np.einsum('bhqk,bhkd->bhqd', probs, v)
    return ctx.transpose(0, 2, 1, 3).reshape(Bq, 1, HID).astype(np.float32)


def _is_lean(inputs):
    """True when biases are zero, mask is all-ones and LN affine is trivial."""
    z = lambda a: not np.any(np.asarray(a))
    return (z(inputs["bq"]) and z(inputs["bqc"]) and z(inputs["bk"])
            and z(inputs["bkc"]) and z(inputs["bv"]) and z(inputs["bvc"])
            and z(inputs["v_ln_b"])
            and np.all(np.asarray(inputs["mask"]) == 1.0)
            and np.all(np.asarray(inputs["v_ln_g"]) == 1.0))


# ---------------------------------------------------------------- bass builder

def _build_nc(nb, chunk_b):
    import concourse.bass as bass
    import concourse.bacc as bacc
    import concourse.tile as tile
    from concourse import mybir

    bf16 = mybir.dt.bfloat16
    f32 = mybir.dt.float32
    AF = mybir.ActivationFunctionType
    OP = mybir.AluOpType

    nch = nb // chunk_b
    crows = chunk_b * S
    nsub = crows // 400          # k-projection N=400 sub-chunks

    nc = bacc.Bacc("TRN2", target_bir_lowering=False, debug=False)

    x_d = nc.dram_tensor("x", [nb * S, IN_F], bf16, kind="ExternalInput").ap()
    xq_d = nc.dram_tensor("xq", [IN_F, nb], bf16, kind="ExternalInput").ap()
    wq_d = nc.dram_tensor("wq", [IN_F, H * QLEN], bf16, kind="ExternalInput").ap()
    wqc_d = nc.dram_tensor("wqc", [IN_F, H * QLEN], bf16, kind="ExternalInput").ap()
    wk_d = nc.dram_tensor("wk", [IN_F, H * QLEN], bf16, kind="ExternalInput").ap()
    wkc_d = nc.dram_tensor("wkc", [IN_F, H * QLEN], bf16, kind="ExternalInput").ap()
    wv_d = nc.dram_tensor("wv", [IN_F, HID], bf16, kind="ExternalInput").ap()
    wvc_d = nc.dram_tensor("wvc", [IN_F, HID], bf16, kind="ExternalInput").ap()
    ctxo_d = nc.dram_tensor("ctxo", [nb, H, HID], bf16, kind="ExternalOutput").ap()
    dout_d = nc.dram_tensor("dout", [nch, H * chunk_b], f32,
                            kind="ExternalOutput").ap()

    with tile.TileContext(nc) as tc:
        from contextlib import ExitStack
        with ExitStack() as ctx:
            consts = ctx.enter_context(tc.tile_pool(name="consts", bufs=1))
            xpool = ctx.enter_context(tc.tile_pool(name="xT", bufs=2))
            kpool = ctx.enter_context(tc.tile_pool(name="kT", bufs=2))
            vgpool = ctx.enter_context(tc.tile_pool(name="vg", bufs=2))
            epool = ctx.enter_context(tc.tile_pool(name="e", bufs=2))
            scr = ctx.enter_context(tc.tile_pool(name="scr", bufs=3))
            stats = ctx.enter_context(tc.tile_pool(name="stats", bufs=2))
            ctxp = ctx.enter_context(tc.tile_pool(name="ctxsb", bufs=2))
            qb = ctx.enter_context(tc.tile_pool(name="qblk", bufs=1))
            # PSUM: proj 4 banks + sc 2 + d 1 + ctx 1 = 8
            psproj = ctx.enter_context(tc.tile_pool(name="psproj", bufs=4, space="PSUM"))
            pssc = ctx.enter_context(tc.tile_pool(name="pssc", bufs=2, space="PSUM"))
            psd = ctx.enter_context(tc.tile_pool(name="psd", bufs=1, space="PSUM"))
            psctx = ctx.enter_context(tc.tile_pool(name="psctx", bufs=1, space="PSUM"))

            # ---- constants
            wk = consts.tile([IN_F, 128], bf16, tag="wk")
            wkc = consts.tile([IN_F, 128], bf16, tag="wkc")
            wv = consts.tile([IN_F, HID], bf16, tag="wv")
            wvc = consts.tile([IN_F, HID], bf16, tag="wvc")
            wq = consts.tile([IN_F, 128], bf16, tag="wq")
            wqc = consts.tile([IN_F, 128], bf16, tag="wqc")
            xq = consts.tile([IN_F, nb], bf16, tag="xq")
            nc.sync.dma_start(out=wk, in_=wk_d)
            nc.sync.dma_start(out=wkc, in_=wkc_d)
            nc.sync.dma_start(out=wv, in_=wv_d)
            nc.sync.dma_start(out=wvc, in_=wvc_d)
            nc.sync.dma_start(out=wq, in_=wq_d)
            nc.sync.dma_start(out=wqc, in_=wqc_d)
            nc.sync.dma_start(out=xq, in_=xq_d)

            ones_col = consts.tile([128, 1], bf16, tag="ones")
            nc.vector.memset(ones_col, 1.0)
            eps_col = consts.tile([128, 1], f32, tag="eps")
            nc.vector.memset(eps_col, LN_EPS)

            blkmask = consts.tile([128, H], bf16, tag="blkmask")
            nc.gpsimd.memset(blkmask, 1.0)
            # keep 1 where 0 <= p - 16*j <= 15 else 0
            nc.gpsimd.affine_select(
                out=blkmask, in_=blkmask, compare_op=OP.is_ge, fill=0.0,
                base=0, pattern=[[-QLEN, H]], channel_multiplier=1)
            nc.gpsimd.affine_select(
                out=blkmask, in_=blkmask, compare_op=OP.is_ge, fill=0.0,
                base=QLEN - 1, pattern=[[QLEN, H]], channel_multiplier=-1)

            # ---- q projection (feature-major), fold 1/sqrt(QLEN)=0.25 in
            qps = psproj.tile([128, nb], f32, tag="proj")
            qcps = psproj.tile([128, nb], f32, tag="proj")
            nc.tensor.matmul(qps, lhsT=wq, rhs=xq, start=True, stop=True)
            nc.tensor.matmul(qcps, lhsT=wqc, rhs=xq, start=True, stop=True)
            qsig = scr.tile([128, nb], bf16, tag="qsig")
            nc.scalar.activation(qsig, qcps, AF.Sigmoid)
            qgT = consts.tile([128, nb], f32, tag="qgT")
            nc.vector.scalar_tensor_tensor(
                out=qgT, in0=qps, scalar=0.25, in1=qsig,
                op0=OP.mult, op1=OP.mult)

            # block-diagonal q for the score matmuls
            qblk = qb.tile([128, nb, H], bf16, tag="qblk")
            for b in range(nb):
                nc.vector.tensor_scalar_mul(
                    out=qblk[:, b, :], in0=blkmask, scalar1=qgT[:, b:b + 1])

            # ---- main loop over chunks
            for c in range(nch):
                xT = xpool.tile([IN_F, crows], bf16, tag="xT")
                nc.sync.dma_start_transpose(
                    out=xT, in_=x_d[c * crows:(c + 1) * crows, :])

                # k projection, feature-major [128 f, crows]
                kT = kpool.tile([128, crows], bf16, tag="kT")
                for sub in range(nsub):
                    sl = slice(sub * 400, (sub + 1) * 400)
                    kps = psproj.tile([128, 400], f32, tag="proj")
                    kcps = psproj.tile([128, 400], f32, tag="proj")
                    nc.tensor.matmul(kps, lhsT=wk, rhs=xT[:, sl], start=True, stop=True)
                    nc.tensor.matmul(kcps, lhsT=wkc, rhs=xT[:, sl], start=True, stop=True)
                    ksig = scr.tile([128, 400], bf16, tag="ksig")
                    nc.scalar.activation(ksig, kcps, AF.Sigmoid)
                    nc.vector.tensor_mul(out=kT[:, sl], in0=kps, in1=ksig)

                # v projection row-major per (b, piece), gate + stats
                vg1 = vgpool.tile([128, chunk_b, HID], bf16, tag="vg1")
                vg2 = vgpool.tile([128, chunk_b, HID], bf16, tag="vg2")
                sums = stats.tile([128, 2 * chunk_b], f32, tag="sums")
                ssq = stats.tile([128, 2 * chunk_b], f32, tag="ssq")
                nc.vector.memset(sums, 0.0)
                nc.vector.memset(ssq, 0.0)
                for b in range(chunk_b):
                    for pi, (po, L) in enumerate(((0, 128), (128, 72))):
                        col = pi * chunk_b + b
                        xsl = xT[:, b * S + po: b * S + po + L]
                        vps = psproj.tile([128, HID], f32, tag="proj")
                        vcps = psproj.tile([128, HID], f32, tag="proj")
                        nc.tensor.matmul(vps[0:L, :], lhsT=xsl, rhs=wv,
                                         start=True, stop=True)
                        nc.tensor.matmul(vcps[0:L, :], lhsT=xsl, rhs=wvc,
                                         start=True, stop=True)
                        vsig = scr.tile([128, HID], bf16, tag="vsig")
                        nc.scalar.activation(vsig[0:L, :], vcps[0:L, :], AF.Sigmoid)
                        vg = vg1 if pi == 0 else vg2
                        nc.vector.scalar_tensor_tensor(
                            out=vg[0:L, b, :], in0=vps[0:L, :], scalar=0.0,
                            in1=vsig[0:L, :], op0=OP.add, op1=OP.mult,
                            accum_out=sums[0:L, col:col + 1])
                        sq = scr.tile([128, HID], bf16, tag="sq")
                        nc.scalar.activation(
                            sq[0:L, :], vg[0:L, b, :], AF.Square,
                            accum_out=ssq[0:L, col:col + 1])

                # LayerNorm stats for the whole chunk
                mu = stats.tile([128, 2 * chunk_b], f32, tag="mu")
                mu2 = stats.tile([128, 2 * chunk_b], f32, tag="mu2")
                var = stats.tile([128, 2 * chunk_b], f32, tag="var")
                rstd = stats.tile([128, 2 * chunk_b], f32, tag="rstd")
                nc.vector.tensor_scalar_mul(out=mu, in0=sums, scalar1=1.0 / HID)
                nc.vector.tensor_mul(out=mu2, in0=mu, in1=mu)
                nc.vector.scalar_tensor_tensor(
                    out=var, in0=ssq, scalar=1.0 / HID, in1=mu2,
                    op0=OP.mult, op1=OP.subtract)
                nc.scalar.activation(rstd, var, AF.Sqrt, bias=eps_col)
                nc.vector.reciprocal(out=rstd, in_=rstd)

                # normalize vg in place: (vg - mu) * rstd
                for b in range(chunk_b):
                    for pi, (po, L) in enumerate(((0, 128), (128, 72))):
                        col = pi * chunk_b + b
                        vg = vg1 if pi == 0 else vg2
                        nc.vector.tensor_scalar(
                            out=vg[0:L, b, :], in0=vg[0:L, b, :],
                            scalar1=mu[0:L, col:col + 1],
                            scalar2=rstd[0:L, col:col + 1],
                            op0=OP.subtract, op1=OP.mult)

                # scores (transposed): [s, 8] per b packed into [*, 8*chunk_b]
                sc1 = pssc.tile([128, H * chunk_b], f32, tag="sc")
                sc2 = pssc.tile([128, H * chunk_b], f32, tag="sc")
                for b in range(chunk_b):
                    nc.tensor.matmul(
                        sc1[:, H * b:H * (b + 1)],
                        lhsT=kT[:, b * S:b * S + 128],
                        rhs=qblk[:, c * chunk_b + b, :], start=True, stop=True)
                    nc.tensor.matmul(
                        sc2[0:72, H * b:H * (b + 1)],
                        lhsT=kT[:, b * S + 128:b * S + 200],
                        rhs=qblk[:, c * chunk_b + b, :], start=True, stop=True)
                e1 = epool.tile([128, H * chunk_b], bf16, tag="e1")
                e2 = epool.tile([128, H * chunk_b], bf16, tag="e2")
                nc.scalar.activation(e1, sc1, AF.Exp)
                nc.scalar.activation(e2[0:72, :], sc2[0:72, :], AF.Exp)

                # softmax denominators: D[8b+h] = sum_s e
                m = H * chunk_b
                dps = psd.tile([128, 1], f32, tag="d")
                nc.tensor.matmul(dps[0:m, :], lhsT=e1, rhs=ones_col,
                                 start=True, stop=False)
                nc.tensor.matmul(dps[0:m, :], lhsT=e2[0:72, :],
                                 rhs=ones_col[0:72, :], start=False, stop=True)
                dsb = stats.tile([128, 1], f32, tag="dsb")
                nc.scalar.copy(dsb[0:m, :], dps[0:m, :])
                nc.sync.dma_start(out=dout_d[c, :], in_=dsb[0:m, :])

                # ctx: [8, 512] per b, unnormalized
                ctxsb = ctxp.tile([H, chunk_b, HID], bf16, tag="ctxsb")
                for b in range(chunk_b):
                    cps = psctx.tile([H, HID], f32, tag="ctx")
                    nc.tensor.matmul(cps, lhsT=e1[:, H * b:H * (b + 1)],
                                     rhs=vg1[:, b, :], start=True, stop=False)
                    nc.tensor.matmul(cps, lhsT=e2[0:72, H * b:H * (b + 1)],
                                     rhs=vg2[0:72, b, :], start=False, stop=True)
                    if b % 2 == 0:
                        nc.scalar.copy(ctxsb[:, b, :], cps)
                    else:
                        nc.vector.tensor_copy(out=ctxsb[:, b, :], in_=cps)
                nc.sync.dma_start(
                    out=ctxo_d[c * chunk_b:(c + 1) * chunk_b, :, :].rearrange(
                        "b h f -> h b f"),
                    in_=ctxsb)

    nc.finalize()
    return nc


# ---------------------------------------------------------------- host driver

def _prep_core_inputs(inputs, nb, n_cores):
    """Build per-core in_maps (bf16 x, xq, weights)."""
    import ml_dtypes
    posid = np.asarray(inputs["posid"])
    if posid.dtype != np.int64 and posid.dtype != np.int32:
        posid = posid.astype(np.int32)
    qcv = np.asarray(inputs["qcv"], dtype=np.float32)
    posembed_bf = _to_bf16(np.asarray(inputs["posembed"], dtype=np.float32))

    ntot = posid.shape[0] * posid.shape[1]
    x = np.empty((ntot, IN_F), dtype=ml_dtypes.bfloat16)
    x[:, :INQ] = _to_bf16(qcv.reshape(ntot, INQ))
    x[:, INQ:] = posembed_bf[posid.reshape(ntot)]

    w = {n: np.ascontiguousarray(_to_bf16(np.asarray(inputs[k], np.float32)))
         for n, k in (("wq", "Wq"), ("wqc", "Wqc"), ("wk", "Wk"),
                      ("wkc", "Wkc"), ("wv", "Wv"), ("wvc", "Wvc"))}

    rows = nb * S
    in_maps = []
    for core in range(n_cores):
        xc = x[core * rows:(core + 1) * rows]
        xqc = np.ascontiguousarray(xc[0::S][:nb].T)    # [128, nb]
        m = {"x": xc, "xq": xqc}
        m.update(w)
        in_maps.append(m)
    return in_maps


def _run_device(inputs):
    from concourse.bass_utils import run_bass_kernel_spmd

    key = "nc"
    if key not in _CACHE:
        _CACHE[key] = _build_nc(NB, CHUNK_B)
    nc = _CACHE[key]

    in_maps = _prep_core_inputs(inputs, NB, N_CORES)
    res = run_bass_kernel_spmd(nc, in_maps, core_ids=list(range(N_CORES)))

    outs = []
    for core in range(N_CORES):
        r = res.results[core]
        ctxo = np.asarray(r["ctxo"], dtype=np.float32)       # [nb, 8, 512]
        d = np.asarray(r["dout"], dtype=np.float32)          # [nch, 8*chunk_b]
        d = d.reshape(NCH, CHUNK_B, H).reshape(NB, H)
        hh = np.arange(H)
        diag = ctxo.reshape(NB, H, H, VLEN)[:, hh, hh, :]    # [nb, H, VLEN]
        ctx = diag / d[:, :, None]
        outs.append(ctx.reshape(NB, 1, HID))
    return np.concatenate(outs, axis=0).astype(np.float32)


def kernel(**inputs) -> np.ndarray:
    args = {k: np.asarray(v) for k, v in inputs.items()}
    for k, v in args.items():
        if v.dtype == np.float64:
            args[k] = v.astype(np.float32)
    if not _is_lean(args):
        return _forward_np(**args)
    try:
        return _run_device(args)
    except Exception:
        import traceback
        traceback.print_exc()
        return _forward_np(**args)
